# revision 10
# baseline (speedup 1.0000x reference)
"""Trainium2 Bass kernel for the MHA+LayerNorm block (B=4, S=2048, D=1024, H=16).

Sharding: 8 cores = (batch b, query-half). Each core computes all 16 heads for
its 1024 query rows: QKV projections, masked softmax, context, output
projection, residual + LayerNorm — fully local, no collectives.

Device layout: scores are computed transposed (S^T = [k, q]) so that
 - the mask bias is injected into the scores PSUM by an identity matmul
   (PE) instead of a DVE elementwise pass,
 - the softmax row-sum falls out of the PV matmul via a ones-augmented V,
 - PV needs no transposes.
attn is written to HBM with a transposed access pattern (512B bursts).
"""

import numpy as np
import ml_dtypes

import concourse.bass as bass
import concourse.tile as tile
import concourse.bacc as bacc
from concourse import mybir
from concourse import bass_utils

FP32 = mybir.dt.float32
FP32R = mybir.dt.float32r
BF16 = mybir.dt.bfloat16

B, S, D, H, DK = 4, 2048, 1024, 16, 64
NQ = S // 2            # query rows per core
QW = 512               # q tile width
NQT = NQ // QW         # q tiles per core (2)
KB = S // 128          # k blocks (16)
NEG = -32768.0         # mask bias (bf16-exact; exp((s+NEG)/8) underflows to 0)
EPS = 1e-5

AF = mybir.ActivationFunctionType
ALU = mybir.AluOpType
AX = mybir.AxisListType

_cache = {}


def _build():
    if "nc" in _cache:
        return _cache["nc"]

    nc = bacc.Bacc("TRN2", target_bir_lowering=False, debug=False, num_devices=8)

    def din(name, shape, dt):
        return nc.dram_tensor(name, list(shape), dt, kind="ExternalInput").ap()

    def dout(name, shape, dt):
        return nc.dram_tensor(name, list(shape), dt, kind="ExternalOutput").ap()

    xqt = din("xqt", [D, NQ], FP32R)     # input_Q[b, qrows].T
    xkt = din("xkt", [D, S], FP32R)      # input_K[b].T
    xvt = din("xvt", [D, S], FP32R)      # input_V[b].T
    wqt = din("wqt", [D, D], FP32R)      # W_Q.T
    wkt = din("wkt", [D, D], FP32R)
    wvt = din("wvt", [D, D], FP32R)
    wfct = din("wfct", [D, D], FP32R)    # W_fc.T
    mbt = din("mbt", [S, NQ], BF16)      # mask[b, qrows].T * NEG
    xqres = din("xqres", [NQ, D], FP32)  # input_Q[b, qrows] for residual
    idm = din("idm", [128, 128], BF16)   # identity
    ones1 = din("ones1", [1, 128], FP32R)

    attn_o = dout("attn_o", [H, NQ, S], FP32)
    ln_o = dout("ln_o", [NQ, D], FP32)

    with tile.TileContext(nc) as tc:
        with (
            tc.tile_pool(name="persist", bufs=1) as pp,
            tc.tile_pool(name="psc", bufs=3, space="PSUM") as psc,     # scores
            tc.tile_pool(name="pctx", bufs=1, space="PSUM") as pctx,   # context
            tc.tile_pool(name="pmisc", bufs=1, space="PSUM") as pmisc, # recip bcast
            tc.tile_pool(name="pout", bufs=2, space="PSUM") as pout,   # outproj
        ):
            kT = pp.tile([128, 8 * S], FP32R)          # [ech 8][2048 k] per 128 e-rows
            v_sb = pp.tile([128, KB * H * 65], BF16)   # per kb: 16 heads x (64 V + ones)
            idm_t = pp.tile([128, 128], BF16)
            nc.sync.dma_start(idm_t[:], idm[:])
            ones_t = pp.tile([1, 128], FP32R)
            nc.sync.dma_start(ones_t[:], ones1[:])
            eps_t = pp.tile([128, 1], FP32)
            nc.gpsimd.memset(eps_t[:], EPS)

            # ones columns of v_sb (before V copies land; disjoint but same tile)
            v3 = v_sb[:].rearrange("p (n c) -> p n c", c=65)
            nc.gpsimd.memset(v3[:, :, 64:65], 1.0)

            # ---------------- K projection ----------------
            with (
                tc.tile_pool(name="wres", bufs=1) as wpool,
                tc.tile_pool(name="xs", bufs=2) as xpool,
            ):
                wk = wpool.tile([128, 8 * D], FP32R, tag="w")
                nc.sync.dma_start(
                    wk[:].rearrange("p (dc e) -> p dc e", dc=8),
                    wkt.rearrange("(dc p) e -> p dc e", p=128),
                )
                for ks in range(4):  # k slabs of 512
                    xk = xpool.tile([128, 8 * 512], FP32R, tag="x")
                    nc.sync.dma_start(
                        xk[:].rearrange("p (dc k) -> p dc k", dc=8),
                        xkt[:, ks * 512:(ks + 1) * 512].rearrange("(dc p) k -> p dc k", p=128),
                    )
                    for ech in range(8):
                        ps = psc.tile([128, 512], FP32, tag="sc")
                        for dch in range(8):
                            nc.tensor.matmul(
                                ps[:],
                                wk[:, dch * D + ech * 128:dch * D + ech * 128 + 128],
                                xk[:, dch * 512:(dch + 1) * 512],
                                start=(dch == 0), stop=(dch == 7),
                            )
                        nc.scalar.copy(kT[:, ech * S + ks * 512:ech * S + ks * 512 + 512], ps[:])

            # ---------------- V projection ----------------
            with (
                tc.tile_pool(name="wres2", bufs=1) as wpool,
                tc.tile_pool(name="xs2", bufs=3) as xpool,
            ):
                wv = wpool.tile([128, 8 * D], FP32R, tag="w")
                nc.sync.dma_start(
                    wv[:].rearrange("p (dc e) -> p dc e", dc=8),
                    wvt.rearrange("(dc p) e -> p dc e", p=128),
                )
                for kch in range(KB):
                    xv = xpool.tile([128, 8 * 128], FP32R, tag="x")
                    nc.sync.dma_start(
                        xv[:].rearrange("p (dc k) -> p dc k", dc=8),
                        xvt[:, kch * 128:(kch + 1) * 128].rearrange("(dc p) k -> p dc k", p=128),
                    )
                    for eh in range(2):
                        ps = psc.tile([128, 512], FP32, tag="sc")
                        for dch in range(8):
                            nc.tensor.matmul(
                                ps[:],
                                xv[:, dch * 128:(dch + 1) * 128],
                                wv[:, dch * D + eh * 512:dch * D + eh * 512 + 512],
                                start=(dch == 0), stop=(dch == 7),
                            )
                        # psum [128, 8 heads x 64] -> v_sb strided (65 pitch)
                        base = kch * (H * 65) + eh * 8 * 65
                        dst = v_sb[:, base:base + 8 * 65].rearrange("p (h c) -> p h c", c=65)[:, :, 0:64]
                        nc.vector.tensor_copy(dst, ps[:].rearrange("p (h d) -> p h d", d=64))

            # ---------------- per q-tile ----------------
            for qt in range(NQT):
                q0 = qt * QW
                with tc.tile_pool(name=f"qt{qt}", bufs=1) as qtp:
                    qT = qtp.tile([128, 8 * QW], FP32R)      # [ech 8][512 q]
                    ctx_sb = qtp.tile([128, 8 * QW], FP32R)  # [hp 8][512 q]

                    # ---- Q projection for this q tile ----
                    with (
                        tc.tile_pool(name="wres3", bufs=1) as wpool,
                        tc.tile_pool(name="xs3", bufs=2) as xpool,
                    ):
                        wq = wpool.tile([128, 8 * D], FP32R, tag="w")
                        nc.sync.dma_start(
                            wq[:].rearrange("p (dc e) -> p dc e", dc=8),
                            wqt.rearrange("(dc p) e -> p dc e", p=128),
                        )
                        xq = xpool.tile([128, 8 * QW], FP32R, tag="x")
                        nc.sync.dma_start(
                            xq[:].rearrange("p (dc k) -> p dc k", dc=8),
                            xqt[:, q0:q0 + QW].rearrange("(dc p) k -> p dc k", p=128),
                        )
                        for ech in range(8):
                            ps = psc.tile([128, QW], FP32, tag="sc")
                            for dch in range(8):
                                nc.tensor.matmul(
                                    ps[:],
                                    wq[:, dch * D + ech * 128:dch * D + ech * 128 + 128],
                                    xq[:, dch * QW:(dch + 1) * QW],
                                    start=(dch == 0), stop=(dch == 7),
                                )
                            nc.scalar.copy(qT[:, ech * QW:(ech + 1) * QW], ps[:])

                    # ---- attention ----
                    with (
                        tc.tile_pool(name="mb", bufs=1) as mbp,
                        tc.tile_pool(name="p0", bufs=1) as p0p,
                        tc.tile_pool(name="stg", bufs=4) as stgp,
                        tc.tile_pool(name="rb", bufs=2) as rbp,
                        tc.tile_pool(name="sm", bufs=4) as smp,
                    ):
                        mb = mbp.tile([128, KB * QW], BF16)
                        nc.sync.dma_start(
                            mb[:].rearrange("p (kb q) -> p kb q", kb=KB),
                            mbt[:, q0:q0 + QW].rearrange("(kb p) q -> p kb q", p=128),
                        )
                        for hp in range(8):
                            hA, hB = 2 * hp, 2 * hp + 1
                            p0A = p0p.tile([128, KB * QW], BF16, tag="p0A")
                            p0B = p0p.tile([128, KB * QW], BF16, tag="p0B")
                            ctxA = pctx.tile([128, QW], FP32, tag="ctxA")
                            ctxB = pctx.tile([128, QW], FP32, tag="ctxB")
                            for kb in range(KB):
                                psA = psc.tile([128, QW], FP32, tag="sc")
                                psB = psc.tile([128, QW], FP32, tag="sc")
                                mbk = mb[:, kb * QW:(kb + 1) * QW]
                                nc.tensor.matmul(psA[:], idm_t[:], mbk, start=True, stop=False)
                                nc.tensor.matmul(psB[:], idm_t[:], mbk, start=True, stop=False)
                                kA = kT[0:64, hp * S + kb * 128:hp * S + kb * 128 + 128]
                                kB_ = kT[64:128, hp * S + kb * 128:hp * S + kb * 128 + 128]
                                nc.tensor.matmul(psA[:], kA, qT[0:64, hp * QW:(hp + 1) * QW],
                                                 start=False, stop=True,
                                                 tile_position=(0, 0), skip_group_check=True)
                                nc.tensor.matmul(psB[:], kB_, qT[64:128, hp * QW:(hp + 1) * QW],
                                                 start=False, stop=True,
                                                 tile_position=(64, 0), skip_group_check=True)
                                nc.scalar.activation(p0A[:, kb * QW:(kb + 1) * QW], psA[:], AF.Exp, bias=0.0, scale=0.125)
                                nc.scalar.activation(p0B[:, kb * QW:(kb + 1) * QW], psB[:], AF.Exp, bias=0.0, scale=0.125)
                                vA = v_sb[:, kb * (H * 65) + hA * 65:kb * (H * 65) + hA * 65 + 65]
                                vB = v_sb[:, kb * (H * 65) + hB * 65:kb * (H * 65) + hB * 65 + 65]
                                nc.tensor.matmul(ctxA[0:65, :], vA, p0A[:, kb * QW:(kb + 1) * QW],
                                                 start=(kb == 0), stop=(kb == KB - 1))
                                nc.tensor.matmul(ctxB[0:65, :], vB, p0B[:, kb * QW:(kb + 1) * QW],
                                                 start=(kb == 0), stop=(kb == KB - 1))
                            for head, ctx, p0, erow in ((hA, ctxA, p0A, 0), (hB, ctxB, p0B, 64)):
                                recip = smp.tile([1, QW], FP32R, tag="recip")
                                with nc.allow_low_precision(reason="fp32r recip for PE broadcast"):
                                    nc.vector.reciprocal(recip[:], ctx[64:65, :])
                                rbps = pmisc.tile([128, QW], FP32, tag="rb")
                                nc.tensor.matmul(rbps[:], ones_t[:], recip[:], start=True, stop=True)
                                rb = rbp.tile([128, QW], FP32, tag="rb")
                                nc.vector.tensor_copy(rb[:], rbps[:])
                                # normalized context -> ctx_sb rows (head A: 0-63, B: 64-127)
                                nc.vector.tensor_mul(
                                    ctx_sb[erow:erow + 64, hp * QW:(hp + 1) * QW],
                                    ctx[0:64, :], rb[0:64, :])
                                # normalized attn tiles -> HBM (transposed write)
                                for kb in range(KB):
                                    stg = stgp.tile([128, QW], FP32, tag="stg")
                                    nc.vector.tensor_mul(stg[:], p0[:, kb * QW:(kb + 1) * QW], rb[:])
                                    nc.sync.dma_start(
                                        attn_o[head, q0:q0 + QW, kb * 128:(kb + 1) * 128].rearrange("q k -> k q"),
                                        stg[:],
                                    )

                    # ---- output projection + residual + LayerNorm ----
                    with (
                        tc.tile_pool(name="wres4", bufs=1) as wpool,
                        tc.tile_pool(name="lnp", bufs=2) as lnp,
                        tc.tile_pool(name="lnx", bufs=4) as lnx,
                        tc.tile_pool(name="lnsq", bufs=1) as lnsq,
                        tc.tile_pool(name="lns", bufs=4) as lns,
                    ):
                        wfc = wpool.tile([128, 8 * D], FP32R, tag="w")
                        nc.sync.dma_start(
                            wfc[:].rearrange("p (ec m) -> p ec m", ec=8),
                            wfct.rearrange("(ec p) m -> p ec m", p=128),
                        )
                        var_t = lns.tile([128, 4], FP32)
                        sd_t = lns.tile([128, 4], FP32)
                        rs_t = lns.tile([128, 4], FP32)
                        xs, negmus = [], []
                        for qs in range(4):  # 128-row slabs of this q tile
                            r0 = q0 + qs * 128
                            xres = lnp.tile([128, D], FP32, tag="xres")
                            nc.sync.dma_start(xres[:], xqres[r0:r0 + 128, :])
                            x_sb = lnx.tile([128, D], FP32, tag="x")
                            for dmh in range(2):
                                ps = pout.tile([128, 512], FP32, tag="op")
                                for ech in range(8):
                                    nc.tensor.matmul(
                                        ps[:],
                                        ctx_sb[:, ech * QW + qs * 128:ech * QW + qs * 128 + 128],
                                        wfc[:, ech * D + dmh * 512:ech * D + dmh * 512 + 512],
                                        start=(ech == 0), stop=(ech == 7),
                                    )
                                nc.vector.tensor_add(
                                    x_sb[:, dmh * 512:(dmh + 1) * 512], ps[:],
                                    xres[:, dmh * 512:(dmh + 1) * 512])
                            sums = lns.tile([128, 1], FP32, tag="sums")
                            nc.vector.reduce_sum(sums[:], x_sb[:], axis=AX.X)
                            negmu = lns.tile([128, 1], FP32, tag="negmu")
                            nc.vector.tensor_scalar_mul(negmu[:], sums[:], -1.0 / D)
                            # Square((x - mu)) with the centering folded into the bias
                            sq = lnsq.tile([128, D], FP32, tag="sq")
                            nc.scalar.activation(sq[:], x_sb[:], AF.Square, bias=negmu[:], scale=1.0,
                                                 accum_out=var_t[:, qs:qs + 1])
                            xs.append(x_sb)
                            negmus.append(negmu)
                        # batched sqrt/recip for the 4 slabs
                        nc.scalar.activation(sd_t[:], var_t[:], AF.Sqrt, bias=eps_t[:], scale=1.0 / D)
                        nc.vector.reciprocal(rs_t[:], sd_t[:])
                        for qs in range(4):
                            r0 = q0 + qs * 128
                            out_t = lnp.tile([128, D], FP32, tag="out")
                            nc.vector.tensor_scalar(out_t[:], xs[qs][:], negmus[qs][:], rs_t[:, qs:qs + 1],
                                                    op0=ALU.add, op1=ALU.mult)
                            nc.sync.dma_start(ln_o[r0:r0 + 128, :], out_t[:])

    nc.compile()
    _cache["nc"] = nc
    return nc


def kernel(input_Q, input_K, input_V, attn_mask, W_Q, W_K, W_V, W_fc):
    input_Q = np.asarray(input_Q, dtype=np.float32)
    input_K = np.asarray(input_K, dtype=np.float32)
    input_V = np.asarray(input_V, dtype=np.float32)
    attn_mask = np.asarray(attn_mask)
    W_Q = np.asarray(W_Q, dtype=np.float32)
    W_K = np.asarray(W_K, dtype=np.float32)
    W_V = np.asarray(W_V, dtype=np.float32)
    W_fc = np.asarray(W_fc, dtype=np.float32)

    nc = _build()

    wqt = np.ascontiguousarray(W_Q.T)
    wkt = np.ascontiguousarray(W_K.T)
    wvt = np.ascontiguousarray(W_V.T)
    wfct = np.ascontiguousarray(W_fc.T)
    idm = np.eye(128, dtype=ml_dtypes.bfloat16)
    ones1 = np.ones((1, 128), dtype=np.float32)

    xkt_b = [np.ascontiguousarray(input_K[b].T) for b in range(B)]
    xvt_b = [np.ascontiguousarray(input_V[b].T) for b in range(B)]

    in_maps = []
    for c in range(8):
        b, qh = c // 2, c % 2
        q0 = qh * NQ
        qrows = slice(q0, q0 + NQ)
        mbt = (attn_mask[b, qrows, :].T.astype(np.float32) * NEG).astype(ml_dtypes.bfloat16)
        in_maps.append({
            "xqt": np.ascontiguousarray(input_Q[b, qrows, :].T),
            "xkt": xkt_b[b],
            "xvt": xvt_b[b],
            "wqt": wqt, "wkt": wkt, "wvt": wvt, "wfct": wfct,
            "mbt": np.ascontiguousarray(mbt),
            "xqres": np.ascontiguousarray(input_Q[b, qrows, :]),
            "idm": idm,
            "ones1": ones1,
        })

    res = bass_utils.run_bass_kernel_spmd(nc, in_maps, core_ids=list(range(8)))

    ln = np.empty((B, S, D), dtype=np.float32)
    attn = np.empty((B, H, S, S), dtype=np.float32)
    for c in range(8):
        b, qh = c // 2, c % 2
        q0 = qh * NQ
        out = res.results[c]
        ln[b, q0:q0 + NQ, :] = out["ln_o"]
        attn[b, :, q0:q0 + NQ, :] = out["attn_o"]
    return ln, attn


# revision 16
# speedup vs baseline: 6522.4304x; 6522.4304x over previous
"""Trainium2 Bass kernel for the MHA+LayerNorm block (B=4, S=2048, D=1024, H=16).

Sharding: 8 cores = (batch b, query-half). Each core computes all 16 heads for
its 1024 query rows: QKV projections, masked softmax, context, output
projection, residual + LayerNorm — fully local, no collectives.

Device layout: scores are computed transposed (S^T = [k, q]) so that
 - the mask bias is injected into the scores PSUM by an identity matmul
   (PE) instead of a DVE elementwise pass,
 - the softmax row-sum falls out of the PV matmul via a ones-augmented V,
 - PV needs no transposes.
attn is written to HBM with a transposed access pattern (512B bursts).
"""

import os
import numpy as np
import ml_dtypes

import jax

try:
    os.makedirs("/tmp/jax_cache_mha", exist_ok=True)
    jax.config.update("jax_compilation_cache_dir", "/tmp/jax_cache_mha")
    jax.config.update("jax_persistent_cache_min_compile_time_secs", 0.0)
    jax.config.update("jax_persistent_cache_min_entry_size_bytes", 0)
except Exception:
    pass

import concourse.bass as bass
import concourse.tile as tile
import concourse.bacc as bacc
from concourse import mybir
from concourse import bass_utils

FP32 = mybir.dt.float32
FP32R = mybir.dt.float32r
BF16 = mybir.dt.bfloat16
FP16 = mybir.dt.float16

B, S, D, H, DK = 4, 2048, 1024, 16, 64
NQ = S // 2            # query rows per core
QW = 512               # q tile width
NQT = NQ // QW         # q tiles per core (2)
KB = S // 128          # k blocks (16)
NEG = -32768.0         # mask bias (bf16-exact; exp((s+NEG)/8) underflows to 0)
EPS = 1e-5

AF = mybir.ActivationFunctionType
ALU = mybir.AluOpType
AX = mybir.AxisListType

_cache = {}


def _build():
    if "nc" in _cache:
        return _cache["nc"]

    nc = bacc.Bacc("TRN2", target_bir_lowering=False, debug=False, num_devices=8)

    def din(name, shape, dt):
        return nc.dram_tensor(name, list(shape), dt, kind="ExternalInput").ap()

    def dout(name, shape, dt):
        return nc.dram_tensor(name, list(shape), dt, kind="ExternalOutput").ap()

    xqt = din("xqt", [D, NQ], FP32R)     # input_Q[b, qrows].T
    xkt = din("xkt", [D, S], FP32R)      # input_K[b].T
    xvt = din("xvt", [D, S], FP32R)      # input_V[b].T
    wqt = din("wqt", [D, D], FP32R)      # W_Q.T
    wkt = din("wkt", [D, D], FP32R)
    wvt = din("wvt", [D, D], FP32R)
    wfct = din("wfct", [D, D], FP32R)    # W_fc.T
    mbt = din("mbt", [S, NQ], FP16)      # mask[b, qrows].T * NEG
    xqres = din("xqres", [NQ, D], FP32)  # input_Q[b, qrows] for residual
    idm = din("idm", [128, 128], FP16)   # identity
    ones1 = din("ones1", [1, 128], FP32R)

    attn_o = dout("attn_o", [H, NQ, S], FP32)
    ln_o = dout("ln_o", [NQ, D], FP32)

    with tile.TileContext(nc) as tc:
        with (
            tc.tile_pool(name="persist", bufs=1) as pp,
            tc.tile_pool(name="psc", bufs=3, space="PSUM") as psc,     # scores
            tc.tile_pool(name="pctx", bufs=1, space="PSUM") as pctx,   # context
            tc.tile_pool(name="pmisc", bufs=1, space="PSUM") as pmisc, # recip bcast
            tc.tile_pool(name="pout", bufs=2, space="PSUM") as pout,   # outproj
        ):
            kT = pp.tile([128, 8 * S], FP32R)          # [ech 8][2048 k] per 128 e-rows
            v_sb = pp.tile([128, KB * H * 65], FP16)   # per kb: 16 heads x (64 V + ones)
            idm_t = pp.tile([128, 128], FP16)
            nc.sync.dma_start(idm_t[:], idm[:])
            ones_t = pp.tile([1, 128], FP32R)
            nc.sync.dma_start(ones_t[:], ones1[:])
            eps_t = pp.tile([128, 1], FP32)
            nc.gpsimd.memset(eps_t[:], EPS)

            # ones columns of v_sb (before V copies land; disjoint but same tile)
            v3 = v_sb[:].rearrange("p (n c) -> p n c", c=65)
            nc.gpsimd.memset(v3[:, :, 64:65], 1.0)

            # ---------------- K projection ----------------
            with (
                tc.tile_pool(name="wres", bufs=1) as wpool,
                tc.tile_pool(name="xs", bufs=2) as xpool,
            ):
                wk = wpool.tile([128, 8 * D], FP32R, tag="w")
                nc.sync.dma_start(
                    wk[:].rearrange("p (dc e) -> p dc e", dc=8),
                    wkt.rearrange("(dc p) e -> p dc e", p=128),
                )
                for ks in range(4):  # k slabs of 512
                    xk = xpool.tile([128, 8 * 512], FP32R, tag="x")
                    nc.sync.dma_start(
                        xk[:].rearrange("p (dc k) -> p dc k", dc=8),
                        xkt[:, ks * 512:(ks + 1) * 512].rearrange("(dc p) k -> p dc k", p=128),
                    )
                    for ech in range(8):
                        ps = psc.tile([128, 512], FP32, tag="sc")
                        for dch in range(8):
                            nc.tensor.matmul(
                                ps[:],
                                wk[:, dch * D + ech * 128:dch * D + ech * 128 + 128],
                                xk[:, dch * 512:(dch + 1) * 512],
                                start=(dch == 0), stop=(dch == 7),
                            )
                        nc.scalar.copy(kT[:, ech * S + ks * 512:ech * S + ks * 512 + 512], ps[:])

            # ---------------- V projection ----------------
            with (
                tc.tile_pool(name="wres2", bufs=1) as wpool,
                tc.tile_pool(name="xs2", bufs=3) as xpool,
            ):
                wv = wpool.tile([128, 8 * D], FP32R, tag="w")
                nc.sync.dma_start(
                    wv[:].rearrange("p (dc e) -> p dc e", dc=8),
                    wvt.rearrange("(dc p) e -> p dc e", p=128),
                )
                for kch in range(KB):
                    xv = xpool.tile([128, 8 * 128], FP32R, tag="x")
                    nc.sync.dma_start(
                        xv[:].rearrange("p (dc k) -> p dc k", dc=8),
                        xvt[:, kch * 128:(kch + 1) * 128].rearrange("(dc p) k -> p dc k", p=128),
                    )
                    for eh in range(2):
                        ps = psc.tile([128, 512], FP32, tag="sc")
                        for dch in range(8):
                            nc.tensor.matmul(
                                ps[:],
                                xv[:, dch * 128:(dch + 1) * 128],
                                wv[:, dch * D + eh * 512:dch * D + eh * 512 + 512],
                                start=(dch == 0), stop=(dch == 7),
                            )
                        # psum [128, 8 heads x 64] -> v_sb strided (65 pitch)
                        base = kch * (H * 65) + eh * 8 * 65
                        dst = v_sb[:, base:base + 8 * 65].rearrange("p (h c) -> p h c", c=65)[:, :, 0:64]
                        nc.vector.tensor_copy(dst, ps[:].rearrange("p (h d) -> p h d", d=64))

            # ---------------- per q-tile ----------------
            for qt in range(NQT):
                q0 = qt * QW
                with tc.tile_pool(name=f"qt{qt}", bufs=1) as qtp:
                    qT = qtp.tile([128, 8 * QW], FP32R)      # [ech 8][512 q]
                    ctx_sb = qtp.tile([128, 8 * QW], FP32R)  # [hp 8][512 q]

                    # ---- Q projection for this q tile ----
                    with (
                        tc.tile_pool(name="wres3", bufs=1) as wpool,
                        tc.tile_pool(name="xs3", bufs=2) as xpool,
                    ):
                        wq = wpool.tile([128, 8 * D], FP32R, tag="w")
                        nc.sync.dma_start(
                            wq[:].rearrange("p (dc e) -> p dc e", dc=8),
                            wqt.rearrange("(dc p) e -> p dc e", p=128),
                        )
                        xq = xpool.tile([128, 8 * QW], FP32R, tag="x")
                        nc.sync.dma_start(
                            xq[:].rearrange("p (dc k) -> p dc k", dc=8),
                            xqt[:, q0:q0 + QW].rearrange("(dc p) k -> p dc k", p=128),
                        )
                        for ech in range(8):
                            ps = psc.tile([128, QW], FP32, tag="sc")
                            for dch in range(8):
                                nc.tensor.matmul(
                                    ps[:],
                                    wq[:, dch * D + ech * 128:dch * D + ech * 128 + 128],
                                    xq[:, dch * QW:(dch + 1) * QW],
                                    start=(dch == 0), stop=(dch == 7),
                                )
                            nc.scalar.copy(qT[:, ech * QW:(ech + 1) * QW], ps[:])

                    # ---- attention ----
                    with (
                        tc.tile_pool(name="mb", bufs=1) as mbp,
                        tc.tile_pool(name="p0", bufs=1) as p0p,
                        tc.tile_pool(name="stg", bufs=4) as stgp,
                        tc.tile_pool(name="rb", bufs=2) as rbp,
                        tc.tile_pool(name="sm", bufs=4) as smp,
                    ):
                        mb = mbp.tile([128, KB * QW], FP16)
                        nc.sync.dma_start(
                            mb[:].rearrange("p (kb q) -> p kb q", kb=KB),
                            mbt[:, q0:q0 + QW].rearrange("(kb p) q -> p kb q", p=128),
                        )
                        for hp in range(8):
                            hA, hB = 2 * hp, 2 * hp + 1
                            p0A = p0p.tile([128, KB * QW], FP16, tag="p0A")
                            p0B = p0p.tile([128, KB * QW], FP16, tag="p0B")
                            ctxA = pctx.tile([128, QW], FP32, tag="ctxA")
                            ctxB = pctx.tile([128, QW], FP32, tag="ctxB")
                            for kb in range(KB):
                                psA = psc.tile([128, QW], FP32, tag="sc")
                                psB = psc.tile([128, QW], FP32, tag="sc")
                                mbk = mb[:, kb * QW:(kb + 1) * QW]
                                nc.tensor.matmul(psA[:], idm_t[:], mbk, start=True, stop=False)
                                nc.tensor.matmul(psB[:], idm_t[:], mbk, start=True, stop=False)
                                kA = kT[0:64, hp * S + kb * 128:hp * S + kb * 128 + 128]
                                kB_ = kT[64:128, hp * S + kb * 128:hp * S + kb * 128 + 128]
                                nc.tensor.matmul(psA[:], kA, qT[0:64, hp * QW:(hp + 1) * QW],
                                                 start=False, stop=True,
                                                 tile_position=(0, 0), skip_group_check=True)
                                nc.tensor.matmul(psB[:], kB_, qT[64:128, hp * QW:(hp + 1) * QW],
                                                 start=False, stop=True,
                                                 tile_position=(64, 0), skip_group_check=True)
                                nc.scalar.activation(p0A[:, kb * QW:(kb + 1) * QW], psA[:], AF.Exp, bias=0.0, scale=0.125)
                                nc.scalar.activation(p0B[:, kb * QW:(kb + 1) * QW], psB[:], AF.Exp, bias=0.0, scale=0.125)
                                vA = v_sb[:, kb * (H * 65) + hA * 65:kb * (H * 65) + hA * 65 + 65]
                                vB = v_sb[:, kb * (H * 65) + hB * 65:kb * (H * 65) + hB * 65 + 65]
                                nc.tensor.matmul(ctxA[0:65, :], vA, p0A[:, kb * QW:(kb + 1) * QW],
                                                 start=(kb == 0), stop=(kb == KB - 1))
                                nc.tensor.matmul(ctxB[0:65, :], vB, p0B[:, kb * QW:(kb + 1) * QW],
                                                 start=(kb == 0), stop=(kb == KB - 1))
                            for head, ctx, p0, erow in ((hA, ctxA, p0A, 0), (hB, ctxB, p0B, 64)):
                                recip = smp.tile([1, QW], FP32R, tag="recip")
                                with nc.allow_low_precision(reason="fp32r recip for PE broadcast"):
                                    nc.vector.reciprocal(recip[:], ctx[64:65, :])
                                rbps = pmisc.tile([128, QW], FP32, tag="rb")
                                nc.tensor.matmul(rbps[:], ones_t[:], recip[:], start=True, stop=True)
                                rb = rbp.tile([128, QW], FP32, tag="rb")
                                nc.vector.tensor_copy(rb[:], rbps[:])
                                # normalized context -> ctx_sb rows (head A: 0-63, B: 64-127)
                                nc.vector.tensor_mul(
                                    ctx_sb[erow:erow + 64, hp * QW:(hp + 1) * QW],
                                    ctx[0:64, :], rb[0:64, :])
                                # normalized attn tiles -> HBM (transposed write)
                                for kb in range(KB):
                                    stg = stgp.tile([128, QW], FP32, tag="stg")
                                    nc.vector.tensor_mul(stg[:], p0[:, kb * QW:(kb + 1) * QW], rb[:])
                                    nc.sync.dma_start(
                                        attn_o[head, q0:q0 + QW, kb * 128:(kb + 1) * 128].rearrange("q k -> k q"),
                                        stg[:],
                                    )

                    # ---- output projection + residual + LayerNorm ----
                    with (
                        tc.tile_pool(name="wres4", bufs=1) as wpool,
                        tc.tile_pool(name="lnp", bufs=2) as lnp,
                        tc.tile_pool(name="lnx", bufs=4) as lnx,
                        tc.tile_pool(name="lnsq", bufs=1) as lnsq,
                        tc.tile_pool(name="lns", bufs=4) as lns,
                    ):
                        wfc = wpool.tile([128, 8 * D], FP32R, tag="w")
                        nc.sync.dma_start(
                            wfc[:].rearrange("p (ec m) -> p ec m", ec=8),
                            wfct.rearrange("(ec p) m -> p ec m", p=128),
                        )
                        var_t = lns.tile([128, 4], FP32)
                        sd_t = lns.tile([128, 4], FP32)
                        rs_t = lns.tile([128, 4], FP32)
                        xs, negmus = [], []
                        for qs in range(4):  # 128-row slabs of this q tile
                            r0 = q0 + qs * 128
                            xres = lnp.tile([128, D], FP32, tag="xres")
                            nc.sync.dma_start(xres[:], xqres[r0:r0 + 128, :])
                            x_sb = lnx.tile([128, D], FP32, tag="x")
                            for dmh in range(2):
                                ps = pout.tile([128, 512], FP32, tag="op")
                                for ech in range(8):
                                    nc.tensor.matmul(
                                        ps[:],
                                        ctx_sb[:, ech * QW + qs * 128:ech * QW + qs * 128 + 128],
                                        wfc[:, ech * D + dmh * 512:ech * D + dmh * 512 + 512],
                                        start=(ech == 0), stop=(ech == 7),
                                    )
                                nc.vector.tensor_add(
                                    x_sb[:, dmh * 512:(dmh + 1) * 512], ps[:],
                                    xres[:, dmh * 512:(dmh + 1) * 512])
                            sums = lns.tile([128, 1], FP32, tag="sums")
                            nc.vector.reduce_sum(sums[:], x_sb[:], axis=AX.X)
                            negmu = lns.tile([128, 1], FP32, tag="negmu")
                            nc.vector.tensor_scalar_mul(negmu[:], sums[:], -1.0 / D)
                            # Square((x - mu)) with the centering folded into the bias
                            sq = lnsq.tile([128, D], FP32, tag="sq")
                            nc.scalar.activation(sq[:], x_sb[:], AF.Square, bias=negmu[:], scale=1.0,
                                                 accum_out=var_t[:, qs:qs + 1])
                            xs.append(x_sb)
                            negmus.append(negmu)
                        # batched sqrt/recip for the 4 slabs
                        nc.scalar.activation(sd_t[:], var_t[:], AF.Sqrt, bias=eps_t[:], scale=1.0 / D)
                        nc.vector.reciprocal(rs_t[:], sd_t[:])
                        for qs in range(4):
                            r0 = q0 + qs * 128
                            out_t = lnp.tile([128, D], FP32, tag="out")
                            nc.vector.tensor_scalar(out_t[:], xs[qs][:], negmus[qs][:], rs_t[:, qs:qs + 1],
                                                    op0=ALU.add, op1=ALU.mult)
                            nc.sync.dma_start(ln_o[r0:r0 + 128, :], out_t[:])

    nc.compile()
    _cache["nc"] = nc
    return nc


def _get_runner():
    """Build (once) a cached jitted SPMD runner for the compiled Bass module.

    Mirrors bass2jax.run_bass_via_pjrt's multi-core path, but caches the
    jitted executable across kernel() calls and creates the donated output
    buffers on-device (avoids uploading 1.1 GB of zeros per call)."""
    if "runner" in _cache:
        return _cache["runner"]
    nc = _build()

    import jax.numpy as jnp
    from jax.sharding import Mesh, PartitionSpec, NamedSharding
    from jax.experimental.shard_map import shard_map
    from concourse.bass2jax import _bass_exec_p, install_neuronx_cc_hook, partition_id_tensor

    install_neuronx_cc_hook()

    partition_name = nc.partition_id_tensor.name if nc.partition_id_tensor else None
    in_names, out_names, out_avals = [], [], []
    for alloc in nc.m.functions[0].allocations:
        if not isinstance(alloc, mybir.MemoryLocationSet):
            continue
        name = alloc.memorylocations[0].name
        if alloc.kind == "ExternalInput":
            if name != partition_name:
                in_names.append(name)
        elif alloc.kind == "ExternalOutput":
            out_names.append(name)
            out_avals.append(jax.core.ShapedArray(
                tuple(alloc.tensor_shape), mybir.dt.np(alloc.dtype)))
    n_params = len(in_names)
    n_outs = len(out_names)
    all_names = in_names + out_names
    if partition_name is not None:
        all_names = all_names + [partition_name]

    def _body(*args):
        operands = list(args)
        if partition_name is not None:
            operands.append(partition_id_tensor())
        outs = _bass_exec_p.bind(
            *operands,
            out_avals=tuple(out_avals),
            in_names=tuple(all_names),
            out_names=tuple(out_names),
            lowering_input_output_aliases=(),
            sim_require_finite=True,
            sim_require_nnan=True,
            nc=nc,
        )
        return tuple(outs)

    devices = jax.devices()[:8]
    mesh = Mesh(np.asarray(devices), ("core",))
    donate = tuple(range(n_params, n_params + n_outs))
    sharded = jax.jit(
        shard_map(_body, mesh=mesh,
                  in_specs=(PartitionSpec("core"),) * (n_params + n_outs),
                  out_specs=(PartitionSpec("core"),) * n_outs,
                  check_rep=False),
        donate_argnums=donate, keep_unused=True,
    )

    zero_shardings = tuple(NamedSharding(mesh, PartitionSpec("core")) for _ in range(n_outs))
    zeromaker = jax.jit(
        lambda: tuple(jnp.zeros((8 * a.shape[0], *a.shape[1:]), a.dtype) for a in out_avals),
        out_shardings=zero_shardings,
    )

    runner = (sharded, zeromaker, in_names, out_names, out_avals, n_params)
    _cache["runner"] = runner
    return runner


def kernel(input_Q, input_K, input_V, attn_mask, W_Q, W_K, W_V, W_fc):
    input_Q = np.asarray(input_Q, dtype=np.float32)
    input_K = np.asarray(input_K, dtype=np.float32)
    input_V = np.asarray(input_V, dtype=np.float32)
    attn_mask = np.asarray(attn_mask)
    W_Q = np.asarray(W_Q, dtype=np.float32)
    W_K = np.asarray(W_K, dtype=np.float32)
    W_V = np.asarray(W_V, dtype=np.float32)
    W_fc = np.asarray(W_fc, dtype=np.float32)

    sharded, zeromaker, in_names, out_names, out_avals, n_params = _get_runner()

    wqt = np.ascontiguousarray(W_Q.T)
    wkt = np.ascontiguousarray(W_K.T)
    wvt = np.ascontiguousarray(W_V.T)
    wfct = np.ascontiguousarray(W_fc.T)
    idm = np.eye(128, dtype=np.float16)
    ones1 = np.ones((1, 128), dtype=np.float32)

    xkt_b = [np.ascontiguousarray(input_K[b].T) for b in range(B)]
    xvt_b = [np.ascontiguousarray(input_V[b].T) for b in range(B)]

    in_maps = []
    for c in range(8):
        b, qh = c // 2, c % 2
        q0 = qh * NQ
        qrows = slice(q0, q0 + NQ)
        mbt = (attn_mask[b, qrows, :].T.astype(np.float32) * NEG).astype(np.float16)
        in_maps.append({
            "xqt": np.ascontiguousarray(input_Q[b, qrows, :].T),
            "xkt": xkt_b[b],
            "xvt": xvt_b[b],
            "wqt": wqt, "wkt": wkt, "wvt": wvt, "wfct": wfct,
            "mbt": np.ascontiguousarray(mbt),
            "xqres": np.ascontiguousarray(input_Q[b, qrows, :]),
            "idm": idm,
            "ones1": ones1,
        })

    concat_in = [
        np.concatenate([in_maps[c][name] for c in range(8)], axis=0)
        for name in in_names
    ]
    zeros_dev = zeromaker()
    out_arrs = sharded(*concat_in, *zeros_dev)
    outs = {name: np.asarray(out_arrs[i]).reshape(8, *out_avals[i].shape)
            for i, name in enumerate(out_names)}

    ln = np.empty((B, S, D), dtype=np.float32)
    attn = np.empty((B, H, S, S), dtype=np.float32)
    for c in range(8):
        b, qh = c // 2, c % 2
        q0 = qh * NQ
        ln[b, q0:q0 + NQ, :] = outs["ln_o"][c]
        attn[b, :, q0:q0 + NQ, :] = outs["attn_o"][c]
    return ln, attn


# revision 20
# speedup vs baseline: 85040.5599x; 13.0382x over previous
"""Trainium2 Bass kernel for the MHA+LayerNorm block (B=4, S=2048, D=1024, H=16).

Sharding: 8 cores = (batch b, query-half). Each core computes all 16 heads for
its 1024 query rows: QKV projections, masked softmax, context, output
projection, residual + LayerNorm — fully local, no collectives.

Device layout: scores are computed transposed (S^T = [k, q]) so that
 - the mask bias is injected into the scores PSUM by an identity matmul
   (PE) instead of a DVE elementwise pass,
 - the softmax row-sum falls out of the PV matmul via a ones-augmented V,
 - PV needs no transposes.
attn is written to HBM with a transposed access pattern (512B bursts).
"""

import os
import numpy as np
import ml_dtypes

import jax

try:
    os.makedirs("/tmp/jax_cache_mha", exist_ok=True)
    jax.config.update("jax_compilation_cache_dir", "/tmp/jax_cache_mha")
    jax.config.update("jax_persistent_cache_min_compile_time_secs", 0.0)
    jax.config.update("jax_persistent_cache_min_entry_size_bytes", 0)
except Exception:
    pass

import concourse.bass as bass
import concourse.tile as tile
import concourse.bacc as bacc
from concourse import mybir
from concourse import bass_utils

FP32 = mybir.dt.float32
FP32R = mybir.dt.float32r
BF16 = mybir.dt.bfloat16
FP16 = mybir.dt.float16

B, S, D, H, DK = 4, 2048, 1024, 16, 64
NQ = S // 2            # query rows per core
QW = 512               # q tile width
NQT = NQ // QW         # q tiles per core (2)
KB = S // 128          # k blocks (16)
NEG = -32768.0         # mask bias (bf16-exact; exp((s+NEG)/8) underflows to 0)
EPS = 1e-5

AF = mybir.ActivationFunctionType
ALU = mybir.AluOpType
AX = mybir.AxisListType

_cache = {}


def _build():
    if "nc" in _cache:
        return _cache["nc"]

    nc = bacc.Bacc("TRN2", target_bir_lowering=False, debug=False, num_devices=8)

    def din(name, shape, dt):
        return nc.dram_tensor(name, list(shape), dt, kind="ExternalInput").ap()

    def dout(name, shape, dt):
        return nc.dram_tensor(name, list(shape), dt, kind="ExternalOutput").ap()

    xqt = din("xqt", [D, NQ], FP32R)     # input_Q[b, qrows].T
    xkt = din("xkt", [D, S], FP32R)      # input_K[b].T
    xvt = din("xvt", [D, S], FP32R)      # input_V[b].T
    wqt = din("wqt", [D, D], FP32R)      # W_Q.T
    wkt = din("wkt", [D, D], FP32R)
    wvt = din("wvt", [D, D], FP32R)
    wfct = din("wfct", [D, D], FP32R)    # W_fc.T
    mbt = din("mbt", [S, NQ], FP16)      # mask[b, qrows].T * NEG
    xqres = din("xqres", [NQ, D], FP32)  # input_Q[b, qrows] for residual
    idm = din("idm", [128, 128], FP16)   # identity

    attn_o = dout("attn_o", [H, S, NQ], FP16)  # [head, key, query] - transposed, host swaps
    ln_o = dout("ln_o", [NQ, D], FP32)

    with tile.TileContext(nc) as tc:
        with (
            tc.tile_pool(name="persist", bufs=1) as pp,
            tc.tile_pool(name="psc", bufs=4, space="PSUM") as psc,     # scores
            tc.tile_pool(name="pctx", bufs=3, space="PSUM") as pctx,   # context
            tc.tile_pool(name="pout", bufs=1, space="PSUM") as pout,   # outproj
        ):
            kT = pp.tile([128, 8 * S], FP32R)          # [ech 8][2048 k] per 128 e-rows
            v_sb = pp.tile([128, KB * H * 65], FP16)   # per kb: 16 heads x (64 V + ones)
            idm_t = pp.tile([128, 128], FP16)
            nc.sync.dma_start(idm_t[:], idm[:])
            eps_t = pp.tile([128, 1], FP32)
            nc.gpsimd.memset(eps_t[:], EPS)

            # ones columns of v_sb (before V copies land; disjoint but same tile)
            v3 = v_sb[:].rearrange("p (n c) -> p n c", c=65)
            nc.gpsimd.memset(v3[:, :, 64:65], 1.0)

            # ---------------- K projection ----------------
            with (
                tc.tile_pool(name="wres", bufs=1) as wpool,
                tc.tile_pool(name="xs", bufs=2) as xpool,
            ):
                wk = wpool.tile([128, 8 * D], FP32R, tag="w")
                nc.sync.dma_start(
                    wk[:].rearrange("p (dc e) -> p dc e", dc=8),
                    wkt.rearrange("(dc p) e -> p dc e", p=128),
                )
                for ks in range(4):  # k slabs of 512
                    xk = xpool.tile([128, 8 * 512], FP32R, tag="x")
                    nc.sync.dma_start(
                        xk[:].rearrange("p (dc k) -> p dc k", dc=8),
                        xkt[:, ks * 512:(ks + 1) * 512].rearrange("(dc p) k -> p dc k", p=128),
                    )
                    for ech in range(8):
                        ps = psc.tile([128, 512], FP32, tag="sc")
                        for dch in range(8):
                            nc.tensor.matmul(
                                ps[:],
                                wk[:, dch * D + ech * 128:dch * D + ech * 128 + 128],
                                xk[:, dch * 512:(dch + 1) * 512],
                                start=(dch == 0), stop=(dch == 7),
                            )
                        nc.scalar.copy(kT[:, ech * S + ks * 512:ech * S + ks * 512 + 512], ps[:])

            # ---------------- V projection ----------------
            with (
                tc.tile_pool(name="wres2", bufs=1) as wpool,
                tc.tile_pool(name="xs2", bufs=3) as xpool,
            ):
                wv = wpool.tile([128, 8 * D], FP32R, tag="w")
                nc.sync.dma_start(
                    wv[:].rearrange("p (dc e) -> p dc e", dc=8),
                    wvt.rearrange("(dc p) e -> p dc e", p=128),
                )
                for kch in range(KB):
                    xv = xpool.tile([128, 8 * 128], FP32R, tag="x")
                    nc.sync.dma_start(
                        xv[:].rearrange("p (dc k) -> p dc k", dc=8),
                        xvt[:, kch * 128:(kch + 1) * 128].rearrange("(dc p) k -> p dc k", p=128),
                    )
                    for eh in range(2):
                        ps = psc.tile([128, 512], FP32, tag="sc")
                        for dch in range(8):
                            nc.tensor.matmul(
                                ps[:],
                                xv[:, dch * 128:(dch + 1) * 128],
                                wv[:, dch * D + eh * 512:dch * D + eh * 512 + 512],
                                start=(dch == 0), stop=(dch == 7),
                            )
                        # psum [128, 8 heads x 64] -> v_sb strided (65 pitch)
                        base = kch * (H * 65) + eh * 8 * 65
                        dst = v_sb[:, base:base + 8 * 65].rearrange("p (h c) -> p h c", c=65)[:, :, 0:64]
                        nc.vector.tensor_copy(dst, ps[:].rearrange("p (h d) -> p h d", d=64))

            # ---------------- per q-tile ----------------
            for qt in range(NQT):
                q0 = qt * QW
                with tc.tile_pool(name=f"qt{qt}", bufs=1) as qtp:
                    qT = qtp.tile([128, 8 * QW], FP32R)      # [ech 8][512 q]
                    ctx_sb = qtp.tile([128, 8 * QW], FP32R)  # [hp 8][512 q]

                    # ---- Q projection for this q tile ----
                    with (
                        tc.tile_pool(name="wres3", bufs=1) as wpool,
                        tc.tile_pool(name="xs3", bufs=2) as xpool,
                    ):
                        wq = wpool.tile([128, 8 * D], FP32R, tag="w")
                        nc.sync.dma_start(
                            wq[:].rearrange("p (dc e) -> p dc e", dc=8),
                            wqt.rearrange("(dc p) e -> p dc e", p=128),
                        )
                        xq = xpool.tile([128, 8 * QW], FP32R, tag="x")
                        nc.sync.dma_start(
                            xq[:].rearrange("p (dc k) -> p dc k", dc=8),
                            xqt[:, q0:q0 + QW].rearrange("(dc p) k -> p dc k", p=128),
                        )
                        for ech in range(8):
                            ps = psc.tile([128, QW], FP32, tag="sc")
                            for dch in range(8):
                                nc.tensor.matmul(
                                    ps[:],
                                    wq[:, dch * D + ech * 128:dch * D + ech * 128 + 128],
                                    xq[:, dch * QW:(dch + 1) * QW],
                                    start=(dch == 0), stop=(dch == 7),
                                )
                            nc.scalar.copy(qT[:, ech * QW:(ech + 1) * QW], ps[:])

                    # ---- attention ----
                    with (
                        tc.tile_pool(name="mb", bufs=1) as mbp,
                        tc.tile_pool(name="p0", bufs=3) as p0p,
                        tc.tile_pool(name="stg", bufs=4) as stgp,
                        tc.tile_pool(name="rb", bufs=2) as rbp,
                        tc.tile_pool(name="sm", bufs=2) as smp,
                    ):
                        mb = mbp.tile([128, KB * QW], FP16)
                        nc.sync.dma_start(
                            mb[:].rearrange("p (kb q) -> p kb q", kb=KB),
                            mbt[:, q0:q0 + QW].rearrange("(kb p) q -> p kb q", p=128),
                        )
                        for hp in range(8):
                            hA, hB = 2 * hp, 2 * hp + 1
                            p0A = p0p.tile([128, KB * QW], FP16, tag="p0")
                            p0B = p0p.tile([128, KB * QW], FP16, tag="p0")
                            ctxA = pctx.tile([128, QW], FP32, tag="ctx")
                            ctxB = pctx.tile([128, QW], FP32, tag="ctx")
                            for kb in range(KB):
                                psA = psc.tile([128, QW], FP32, tag="sc")
                                psB = psc.tile([128, QW], FP32, tag="sc")
                                mbk = mb[:, kb * QW:(kb + 1) * QW]
                                nc.tensor.matmul(psA[:], idm_t[:], mbk, start=True, stop=False)
                                nc.tensor.matmul(psB[:], idm_t[:], mbk, start=True, stop=False)
                                kA = kT[0:64, hp * S + kb * 128:hp * S + kb * 128 + 128]
                                kB_ = kT[64:128, hp * S + kb * 128:hp * S + kb * 128 + 128]
                                nc.tensor.matmul(psA[:], kA, qT[0:64, hp * QW:(hp + 1) * QW],
                                                 start=False, stop=True,
                                                 tile_position=(0, 0), skip_group_check=True)
                                nc.tensor.matmul(psB[:], kB_, qT[64:128, hp * QW:(hp + 1) * QW],
                                                 start=False, stop=True,
                                                 tile_position=(64, 0), skip_group_check=True)
                                nc.scalar.activation(p0A[:, kb * QW:(kb + 1) * QW], psA[:], AF.Exp, bias=0.0, scale=0.125)
                                nc.scalar.activation(p0B[:, kb * QW:(kb + 1) * QW], psB[:], AF.Exp, bias=0.0, scale=0.125)
                                vA = v_sb[:, kb * (H * 65) + hA * 65:kb * (H * 65) + hA * 65 + 65]
                                vB = v_sb[:, kb * (H * 65) + hB * 65:kb * (H * 65) + hB * 65 + 65]
                                nc.tensor.matmul(ctxA[0:65, :], vA, p0A[:, kb * QW:(kb + 1) * QW],
                                                 start=(kb == 0), stop=(kb == KB - 1))
                                nc.tensor.matmul(ctxB[0:65, :], vB, p0B[:, kb * QW:(kb + 1) * QW],
                                                 start=(kb == 0), stop=(kb == KB - 1))
                            for head, ctx, p0, erow in ((hA, ctxA, p0A, 0), (hB, ctxB, p0B, 64)):
                                recip = smp.tile([1, QW], FP32, tag="recip")
                                nc.vector.reciprocal(recip[:], ctx[64:65, :])
                                recip16 = smp.tile([1, QW], FP16, tag="recip16")
                                nc.vector.tensor_copy(recip16[:], recip[:])
                                rb = rbp.tile([128, QW], FP32, tag="rb")
                                nc.gpsimd.partition_broadcast(rb[:], recip[:])
                                rb16 = rbp.tile([128, QW], FP16, tag="rb16")
                                nc.gpsimd.partition_broadcast(rb16[:], recip16[:])
                                # normalized context -> ctx_sb rows (head A: 0-63, B: 64-127)
                                nc.vector.tensor_mul(
                                    ctx_sb[erow:erow + 64, hp * QW:(hp + 1) * QW],
                                    ctx[0:64, :], rb[0:64, :])
                                # normalized attn tiles -> HBM ([k, q] layout, contiguous q runs)
                                for kb in range(KB):
                                    stg = stgp.tile([128, QW], FP16, tag="stg")
                                    nc.vector.tensor_mul(stg[:], p0[:, kb * QW:(kb + 1) * QW], rb16[:])
                                    nc.sync.dma_start(
                                        attn_o[head, kb * 128:(kb + 1) * 128, q0:q0 + QW],
                                        stg[:],
                                    )

                    # ---- output projection + residual + LayerNorm ----
                    with (
                        tc.tile_pool(name="wres4", bufs=1) as wpool,
                        tc.tile_pool(name="lnp", bufs=2) as lnp,
                        tc.tile_pool(name="lnx", bufs=4) as lnx,
                        tc.tile_pool(name="lnsq", bufs=1) as lnsq,
                        tc.tile_pool(name="lns", bufs=4) as lns,
                    ):
                        wfc = wpool.tile([128, 8 * D], FP32R, tag="w")
                        nc.sync.dma_start(
                            wfc[:].rearrange("p (ec m) -> p ec m", ec=8),
                            wfct.rearrange("(ec p) m -> p ec m", p=128),
                        )
                        var_t = lns.tile([128, 4], FP32)
                        sd_t = lns.tile([128, 4], FP32)
                        rs_t = lns.tile([128, 4], FP32)
                        xs, negmus = [], []
                        for qs in range(4):  # 128-row slabs of this q tile
                            r0 = q0 + qs * 128
                            xres = lnp.tile([128, D], FP32, tag="xres")
                            nc.sync.dma_start(xres[:], xqres[r0:r0 + 128, :])
                            x_sb = lnx.tile([128, D], FP32, tag="x")
                            for dmh in range(2):
                                ps = pout.tile([128, 512], FP32, tag="op")
                                for ech in range(8):
                                    nc.tensor.matmul(
                                        ps[:],
                                        ctx_sb[:, ech * QW + qs * 128:ech * QW + qs * 128 + 128],
                                        wfc[:, ech * D + dmh * 512:ech * D + dmh * 512 + 512],
                                        start=(ech == 0), stop=(ech == 7),
                                    )
                                nc.vector.tensor_add(
                                    x_sb[:, dmh * 512:(dmh + 1) * 512], ps[:],
                                    xres[:, dmh * 512:(dmh + 1) * 512])
                            sums = lns.tile([128, 1], FP32, tag="sums")
                            nc.vector.reduce_sum(sums[:], x_sb[:], axis=AX.X)
                            negmu = lns.tile([128, 1], FP32, tag="negmu")
                            nc.vector.tensor_scalar_mul(negmu[:], sums[:], -1.0 / D)
                            # Square((x - mu)) with the centering folded into the bias
                            sq = lnsq.tile([128, D], FP32, tag="sq")
                            nc.scalar.activation(sq[:], x_sb[:], AF.Square, bias=negmu[:], scale=1.0,
                                                 accum_out=var_t[:, qs:qs + 1])
                            xs.append(x_sb)
                            negmus.append(negmu)
                        # batched sqrt/recip for the 4 slabs
                        nc.scalar.activation(sd_t[:], var_t[:], AF.Sqrt, bias=eps_t[:], scale=1.0 / D)
                        nc.vector.reciprocal(rs_t[:], sd_t[:])
                        for qs in range(4):
                            r0 = q0 + qs * 128
                            out_t = lnp.tile([128, D], FP32, tag="out")
                            nc.vector.tensor_scalar(out_t[:], xs[qs][:], negmus[qs][:], rs_t[:, qs:qs + 1],
                                                    op0=ALU.add, op1=ALU.mult)
                            nc.sync.dma_start(ln_o[r0:r0 + 128, :], out_t[:])

    nc.compile()
    _cache["nc"] = nc
    return nc


def _get_runner():
    """Build (once) a cached jitted SPMD runner for the compiled Bass module.

    Mirrors bass2jax.run_bass_via_pjrt's multi-core path, but caches the
    jitted executable across kernel() calls and creates the donated output
    buffers on-device (avoids uploading 1.1 GB of zeros per call)."""
    if "runner" in _cache:
        return _cache["runner"]
    nc = _build()

    import jax.numpy as jnp
    from jax.sharding import Mesh, PartitionSpec, NamedSharding
    from jax.experimental.shard_map import shard_map
    from concourse.bass2jax import _bass_exec_p, install_neuronx_cc_hook, partition_id_tensor

    install_neuronx_cc_hook()

    partition_name = nc.partition_id_tensor.name if nc.partition_id_tensor else None
    in_names, out_names, out_avals = [], [], []
    for alloc in nc.m.functions[0].allocations:
        if not isinstance(alloc, mybir.MemoryLocationSet):
            continue
        name = alloc.memorylocations[0].name
        if alloc.kind == "ExternalInput":
            if name != partition_name:
                in_names.append(name)
        elif alloc.kind == "ExternalOutput":
            out_names.append(name)
            out_avals.append(jax.core.ShapedArray(
                tuple(alloc.tensor_shape), mybir.dt.np(alloc.dtype)))
    n_params = len(in_names)
    n_outs = len(out_names)
    all_names = in_names + out_names
    if partition_name is not None:
        all_names = all_names + [partition_name]

    def _body(*args):
        operands = list(args)
        if partition_name is not None:
            operands.append(partition_id_tensor())
        outs = _bass_exec_p.bind(
            *operands,
            out_avals=tuple(out_avals),
            in_names=tuple(all_names),
            out_names=tuple(out_names),
            lowering_input_output_aliases=(),
            sim_require_finite=True,
            sim_require_nnan=True,
            nc=nc,
        )
        return tuple(outs)

    devices = jax.devices()[:8]
    mesh = Mesh(np.asarray(devices), ("core",))
    donate = tuple(range(n_params, n_params + n_outs))
    sharded = jax.jit(
        shard_map(_body, mesh=mesh,
                  in_specs=(PartitionSpec("core"),) * (n_params + n_outs),
                  out_specs=(PartitionSpec("core"),) * n_outs,
                  check_rep=False),
        donate_argnums=donate, keep_unused=True,
    )

    zero_shardings = tuple(NamedSharding(mesh, PartitionSpec("core")) for _ in range(n_outs))
    zeromaker = jax.jit(
        lambda: tuple(jnp.zeros((8 * a.shape[0], *a.shape[1:]), a.dtype) for a in out_avals),
        out_shardings=zero_shardings,
    )

    runner = (sharded, zeromaker, in_names, out_names, out_avals, n_params)
    _cache["runner"] = runner
    return runner


def kernel(input_Q, input_K, input_V, attn_mask, W_Q, W_K, W_V, W_fc):
    input_Q = np.asarray(input_Q, dtype=np.float32)
    input_K = np.asarray(input_K, dtype=np.float32)
    input_V = np.asarray(input_V, dtype=np.float32)
    attn_mask = np.asarray(attn_mask)
    W_Q = np.asarray(W_Q, dtype=np.float32)
    W_K = np.asarray(W_K, dtype=np.float32)
    W_V = np.asarray(W_V, dtype=np.float32)
    W_fc = np.asarray(W_fc, dtype=np.float32)

    sharded, zeromaker, in_names, out_names, out_avals, n_params = _get_runner()

    wqt = np.ascontiguousarray(W_Q.T)
    wkt = np.ascontiguousarray(W_K.T)
    wvt = np.ascontiguousarray(W_V.T)
    wfct = np.ascontiguousarray(W_fc.T)
    idm = np.eye(128, dtype=np.float16)

    xkt_b = [np.ascontiguousarray(input_K[b].T) for b in range(B)]
    xvt_b = [np.ascontiguousarray(input_V[b].T) for b in range(B)]

    in_maps = []
    for c in range(8):
        b, qh = c // 2, c % 2
        q0 = qh * NQ
        qrows = slice(q0, q0 + NQ)
        mbt = (attn_mask[b, qrows, :].T.astype(np.float32) * NEG).astype(np.float16)
        in_maps.append({
            "xqt": np.ascontiguousarray(input_Q[b, qrows, :].T),
            "xkt": xkt_b[b],
            "xvt": xvt_b[b],
            "wqt": wqt, "wkt": wkt, "wvt": wvt, "wfct": wfct,
            "mbt": np.ascontiguousarray(mbt),
            "xqres": np.ascontiguousarray(input_Q[b, qrows, :]),
            "idm": idm,
        })

    concat_in = [
        np.concatenate([in_maps[c][name] for c in range(8)], axis=0)
        for name in in_names
    ]
    zeros_dev = zeromaker()
    out_arrs = sharded(*concat_in, *zeros_dev)
    outs = {name: np.asarray(out_arrs[i]).reshape(8, *out_avals[i].shape)
            for i, name in enumerate(out_names)}

    ln = np.empty((B, S, D), dtype=np.float32)
    attn = np.empty((B, H, S, S), dtype=np.float32)
    for c in range(8):
        b, qh = c // 2, c % 2
        q0 = qh * NQ
        ln[b, q0:q0 + NQ, :] = outs["ln_o"][c]
        # attn_o is [H, key, query] fp16; swap to [H, query, key] and upcast
        attn[b, :, q0:q0 + NQ, :] = outs["attn_o"][c].transpose(0, 2, 1)
    return ln, attn
